# revision 29
# baseline (speedup 1.0000x reference)
"""MetaNetImageEncoder Trainium2 kernel (fp8, v4).

Data-parallel over batch: 8 samples per NeuronCore x 8 cores.

Numerics: x, W1, dW1, dW2, mixed weights nW1 are fp8e4m3. Scales:
  phase 1:  pa = x8 @ (64 W1)                  -> relu(pa/64 + b1)
  phase 3:  pm = (8 c) x (8 dW1) = 64 c dW1;  copy-add w1i(=64 W1)
            -> mxiall = 64 (W1 + M) = 64 nW1   (fp8)
  phase 4:  pf = x8 @ (64 nW1)                 -> relu(pf/64 + nb1)
  phase 5:  po8 = (64 c pooled) @ (64 dW2) = 4096 delta -> * 1/4096
W2, MetaNet, biases, pooled vectors stay bf16/f32.

Engine plan:
  sync    all bulk HBM loads (xt, w1, dw1 x8, w1i, w2, dw2 x8), then
          4 de-interleave DMAs per icl, final store.  No compute.
  scalar  small consts, then pure ACT work (relus) + 2 de-int per icl.
  vector  pool reduces, 3/5 of phase-3 PSUM+w1i add-copies, uall.
  gpsimd  (Pool engine) 2/5 of phase-3 add-copies + 2 de-int per icl.
  tensor  matmuls; phase 1 DoubleRow (FD 392), phase 4 normal fp8
          (FD 196 < 256 so DoubleRow would be LDWEIGHTS-bound),
          phase 5 delta DoubleRow (tiny stationary).
De-interleave DMAs are issued per-icl inside phase 3 so they overlap
the mixing; phase 4 needs no per-sample DMA or W1 add at all.
"""
import numpy as np
import ml_dtypes

import concourse.bass as bass
import concourse.mybir as mybir
import concourse.tile as tile
from concourse.vector_clock import ScopedClock
from concourse.bass_utils import run_bass_kernel_spmd

F32 = mybir.dt.float32
BF16 = mybir.dt.bfloat16
FP8 = mybir.dt.float8e4
DR = mybir.MatmulPerfMode.DoubleRow
RELU = mybir.ActivationFunctionType.Relu
ADD = mybir.AluOpType.add
MULT = mybir.AluOpType.mult
AXX = mybir.AxisListType.X

P = 16
D = 768
T = 8
HM = 192
NPAT = 196          # 14*14 patches
B = 64
NCORES = 8
BC = B // NCORES    # 8 samples per core
NB = BC * NPAT      # 1568
KT = D // 128       # 6 k-tiles
SC = 64.0           # fp8 weight-space scale
SCC = 8.0           # coefficient scale (SCC * (SC/SCC) = SC)

_PATCHED = False


def _apply_tile_patch():
    """This container's walrus allows only one sem wait per instruction;
    TileContext's exit drain attaches one wait per live semaphore. Split
    them onto standalone single-wait nops."""
    global _PATCHED
    if _PATCHED:
        return
    _PATCHED = True

    def _patched(self, tick_clock, wait_clock):
        carrier = self.nc.sync.nop(nofuse=True, hint="drain_waits")
        wait_clock.add_sem_waits(
            carrier.ins, ScopedClock({None: tick_clock.global_clock})
        )
        si = carrier.ins.sync_info
        waits = list(si.on_wait) if si else []
        if len(waits) > 1:
            carrier.ins.sync_info = mybir.SyncInfo(on_wait=[waits[0]], on_update=[])
            for w in waits[1:]:
                extra = self.nc.sync.nop(nofuse=True, hint="drain_waits")
                extra.ins.sync_info = mybir.SyncInfo(on_wait=[w], on_update=[])
        self.nc.sync.drain()
        self.nc.all_engine_barrier()
        popped = self.nc._tile_sem_poison_stack.pop()
        assert popped is self._sem_poison
        self.nc.clear_and_free_semaphores(list(self.sems.allocated().values()))
        self.nc.all_engine_barrier()

    tile.TileContext._drain_and_barrier = _patched


def _split_multi_waits(nc, max_waits: int = 1):
    """Hoist extra sem waits onto same-engine InstNoOp carriers."""
    for f in nc.m.functions:
        for blk in f.blocks:
            out = []
            for inst in blk.instructions:
                si = inst.sync_info
                if si is not None and len(si.on_wait) > max_waits:
                    waits = list(si.on_wait)
                    for i, w in enumerate(waits[:-max_waits]):
                        out.append(mybir.InstNoOp(
                            name=f"{inst.name}-w{i}",
                            sync_info=mybir.SyncInfo(on_wait=[w], on_update=[]),
                            bass_nofuse=True,
                            engine=inst.engine,
                        ))
                    inst.sync_info = mybir.SyncInfo(
                        on_wait=waits[-max_waits:], on_update=list(si.on_update)
                    )
                out.append(inst)
            blk.instructions = out


def build_kernel():
    nc = bass.Bass(target_bir_lowering=False, trn_type="TRN2")

    din = {}
    def inp(name, shape, dt):
        din[name] = nc.dram_tensor(name, shape, dt, kind="ExternalInput")
        return din[name]

    xt = inp("xt", (128, KT, NB), FP8)           # patches^T  [i_local, kt, (b,n)]
    w1 = inp("w1", (128, KT, D), FP8)            # 64*W1 [i_local, kt, j]
    w1i = inp("w1i", (128, 8, KT, D), FP8)       # 64*W1 [(b,s), icl, it, j]
    w2 = inp("w2", (128, KT, D), BF16)           # W2 [j_local, kt, e]
    dw1 = inp("dw1", (48, 128, D), FP8)          # 8*dW1 [icl*6+it, (t,s16), j]
    dw2 = inp("dw2", (T, KT, 128, D), FP8)       # 64*dW2 [t, kt, j_local, e]
    db1 = inp("db1", (T, D), BF16)
    db2 = inp("db2", (T, D), BF16)
    b1t = inp("b1t", (128, KT), F32)             # b1 [j_local, jt]
    b2t = inp("b2t", (128, KT), F32)             # b2 [e_local, et]
    b2r = inp("b2r", (BC, D), F32)               # b2 replicated over samples
    mw1 = inp("mw1", (128, KT, HM), BF16)
    mb1t = inp("mb1t", (128, 2), F32)
    mw2 = inp("mw2", (128, 2, T), BF16)          # [h_local, g, t], g=1 padded
    mb2t = inp("mb2t", (T, 1), F32)
    iexp = inp("iexp", (T, 128), F32)            # repeat(eye(8),16,axis=1)
    mask16 = inp("mask16", (128, P), BF16)       # 8 * (p%16==s')
    selb = inp("selb", (T, T, 128), BF16)        # 64 * (t'==t)

    out = nc.dram_tensor("out", (BC, D), F32, kind="ExternalOutput")

    with tile.TileContext(nc) as tc:
        with (
            tc.tile_pool(name="big", bufs=1) as big,
            tc.tile_pool(name="sm", bufs=1) as sm,
            tc.tile_pool(name="dwp", bufs=10) as dwp,
            tc.tile_pool(name="scr", bufs=2) as scr,
        ):
            # ---------- bulk loads: sync ring only (FIFO orders HBM) -----
            # w1 first, then xt in k-chunks: phase 1 starts on chunk 0.
            w1_sb = big.tile([128, KT, D], FP8, tag="w1")
            nc.sync.dma_start(w1_sb[:], w1[:])
            xt_sb = big.tile([128, KT, NB], FP8, tag="xt")
            for g in range(3):
                nc.sync.dma_start(xt_sb[:, 2 * g:2 * g + 2, :],
                                  xt[:, 2 * g:2 * g + 2, :])
            # w2 right after (phase 2 gate), then dw1, then w1i per-icl
            w2_sb = big.tile([128, KT, D], BF16, tag="w2")
            nc.sync.dma_start(w2_sb[:], w2[:])
            dw1_tiles = []
            for icl in range(8):
                t_ = dwp.tile([128, KT, D], FP8, tag="dw")
                nc.sync.dma_start(
                    t_[:],
                    dw1[icl * KT:(icl + 1) * KT].rearrange("k p j -> p k j"))
                dw1_tiles.append(t_)
            w1i_sb = big.tile([128, 8, KT, D], FP8, tag="w1i")
            for icl in range(8):
                nc.sync.dma_start(w1i_sb[:, icl, :, :], w1i[:, icl, :, :])

            # ---------- small consts: scalar ring ----------
            b1t_sb = sm.tile([128, KT], F32, tag="b1t")
            nc.scalar.dma_start(b1t_sb[:], b1t[:])
            mb1t_sb = sm.tile([128, 2], F32, tag="mb1t")
            nc.scalar.dma_start(mb1t_sb[:], mb1t[:])
            mw1_sb = sm.tile([128, KT, HM], BF16, tag="mw1")
            nc.scalar.dma_start(mw1_sb[:], mw1[:])
            mw2_sb = sm.tile([128, 2, T], BF16, tag="mw2")
            nc.scalar.dma_start(mw2_sb[:], mw2[:])
            mb2t_sb = sm.tile([T, 1], F32, tag="mb2t")
            nc.scalar.dma_start(mb2t_sb[:], mb2t[:])
            b2t_sb = sm.tile([128, KT], F32, tag="b2t")
            nc.scalar.dma_start(b2t_sb[:], b2t[:])
            b2r_sb = sm.tile([BC, D], F32, tag="b2r")
            nc.scalar.dma_start(b2r_sb[:], b2r[:])
            iexp_sb = sm.tile([T, 128], F32, tag="iexp")
            nc.scalar.dma_start(iexp_sb[:], iexp[:])
            mask16_sb = sm.tile([128, P], BF16, tag="mask16")
            nc.scalar.dma_start(mask16_sb[:], mask16[:])
            selb_sb = sm.tile([T, T, 128], BF16, tag="selb")
            nc.scalar.dma_start(selb_sb[:], selb[:])
            db1_sb = sm.tile([T, D], BF16, tag="db1")
            nc.scalar.dma_start(db1_sb[:], db1[:])
            db2_sb = sm.tile([T, D], BF16, tag="db2")
            nc.scalar.dma_start(db2_sb[:], db2[:])

            poolb = sm.tile([128, KT, BC], F32, tag="poolb")
            pooln = sm.tile([128, KT, BC], F32, tag="pooln")

            # ---------- phase 1: base pass (fp8 DoubleRow) ----------
            # g outer / ch inner reuses each stationary across 4 matmuls
            with tc.tile_pool(name="psA", bufs=8, space="PSUM") as psA:
                for jt in range(KT):
                    pas = []
                    for ch in range(4):
                        pa = psA.tile([128, 2, NPAT], F32, tag="a")
                        pas.append(pa)
                    for g in range(3):
                        for ch in range(4):
                            nc.tensor.matmul(
                                pas[ch][:].rearrange("p b n -> p (b n)"),
                                w1_sb[:, 2 * g:2 * g + 2,
                                      jt * 128:(jt + 1) * 128],
                                xt_sb[:, 2 * g:2 * g + 2,
                                      ch * 392:(ch + 1) * 392],
                                start=(g == 0), stop=(g == 2),
                                perf_mode=DR)
                    ro = scr.tile([128, 2 * 4, NPAT], BF16, tag="ro")
                    for ch in range(4):
                        nc.scalar.activation(
                            ro[:, 2 * ch:2 * ch + 2, :], pas[ch][:], RELU,
                            bias=b1t_sb[:, jt:jt + 1], scale=1.0 / SC)
                    nc.vector.tensor_reduce(
                        poolb[:, jt, :], ro[:], axis=AXX, op=ADD)

            # ---------- phase 2: MetaNet (bf16) ----------
            with (
                tc.tile_pool(name="pst", bufs=4, space="PSUM") as pst,
                tc.tile_pool(name="psJ", bufs=2, space="PSUM") as psJ,
            ):
                def warm(n=2):
                    for _ in range(n):
                        pj = psJ.tile([128, 512], F32, tag="j")
                        nc.tensor.matmul(
                            pj[:], w1_sb[:, 0, 0:128], xt_sb[:, 0, 0:512],
                            start=True, stop=True)

                poolb_bf = sm.tile([128, KT, BC], BF16, tag="poolbbf")
                nc.scalar.mul(poolb_bf[:], poolb[:], 1.0 / NPAT)

                # base2^T[e, b] = W2.T @ pooled + b2  (input to MetaNet)
                base2_bf = sm.tile([128, KT * BC], BF16, tag="base2bf")
                for et in range(KT):
                    p2 = pst.tile([128, T], F32, tag="tiny")
                    for kt in range(KT):
                        nc.tensor.matmul(
                            p2[:], w2_sb[:, kt, et * 128:(et + 1) * 128],
                            poolb_bf[:, kt, :],
                            start=(kt == 0), stop=(kt == KT - 1))
                    nc.vector.tensor_scalar_add(
                        base2_bf[:, et * BC:(et + 1) * BC], p2[:],
                        b2t_sb[:, et:et + 1])
                warm()

                mh0 = sm.tile([128, T], BF16, tag="mh0")
                mh1 = sm.tile([64, T], BF16, tag="mh1")
                for g, mh_g in ((0, mh0), (1, mh1)):
                    cols = 128 if g == 0 else 64
                    pm = pst.tile([cols, T], F32, tag="tiny")
                    for kt in range(KT):
                        nc.tensor.matmul(
                            pm[:], mw1_sb[:, kt, g * 128:g * 128 + cols],
                            base2_bf[:, kt * BC:(kt + 1) * BC],
                            start=(kt == 0), stop=(kt == KT - 1))
                    nc.scalar.activation(mh_g[:], pm[:], RELU,
                                         bias=mb1t_sb[:cols, g:g + 1])
                warm()

                pc = pst.tile([T, T], F32, tag="tiny")
                nc.tensor.matmul(pc[:], mw2_sb[:, 0, :], mh0[:],
                                 start=True, stop=False)
                nc.tensor.matmul(pc[:], mw2_sb[0:64, 1, :], mh1[:],
                                 start=False, stop=True)
                coefsT = sm.tile([T, T], F32, tag="coefsT")
                nc.vector.tensor_scalar_add(coefsT[:], pc[:], mb2t_sb[:])
                coefsT_bf = sm.tile([T, T], BF16, tag="coefsTbf")
                nc.vector.tensor_copy(coefsT_bf[:], coefsT[:])
                warm()

                # coefficient replication [128, 8]: cRep[(t,s), b] = c[t, b]
                pr = pst.tile([128, T], F32, tag="tiny")
                nc.tensor.matmul(pr[:], iexp_sb[:], coefsT[:],
                                 start=True, stop=True)
                crep = sm.tile([128, T], F32, tag="crep")
                nc.vector.tensor_copy(crep[:], pr[:])

                # block-diag stationary Cb[(t,s), (b,s')] = 8 c (fp8)
                cb_sb = sm.tile([128, 128], FP8, tag="cb")
                for b in range(BC):
                    nc.vector.tensor_scalar_mul(
                        cb_sb[:, b * P:(b + 1) * P], mask16_sb[:],
                        crep[:, b:b + 1])

                # cbc[p, t, b] = 64 c[b, t] for all partitions (phase 5)
                cbc = sm.tile([128, T, BC], BF16, tag="cbc")
                for t in range(T):
                    pu = pst.tile([128, T], F32, tag="tiny")
                    nc.tensor.matmul(pu[:], selb_sb[:, t, :], coefsT_bf[:],
                                     start=True, stop=True)
                    nc.vector.tensor_copy(cbc[:, t, :], pu[:])
                warm()

                # nb1t[j_local, jt, b] = b1 + coefs @ db1
                nb1t = sm.tile([128, KT, BC], F32, tag="nb1t")
                for jt in range(KT):
                    pb = pst.tile([128, T], F32, tag="tiny")
                    nc.tensor.matmul(pb[:], db1_sb[:, jt * 128:(jt + 1) * 128],
                                     coefsT_bf[:], start=True, stop=True)
                    nc.vector.tensor_scalar_add(
                        nb1t[:, jt, :], pb[:], b1t_sb[:, jt:jt + 1])

            # ---------- phase 3: mixing + W1 fold + de-interleave ----------
            mxiall = big.tile([128, 8, KT, D], FP8, tag="mxiall")
            mxcball = big.tile([128, BC, KT, D], FP8, tag="mxcball")
            with tc.tile_pool(name="psM", bufs=4, space="PSUM") as psM:
                for icl in range(8):
                    dwt6 = dw1_tiles[icl]
                    for it in range(KT):
                        pm2 = psM.tile([128, 2, 512], F32, tag="m")  # 2 banks
                        for jh in range(2):
                            nc.tensor.matmul(
                                pm2[:, jh, 0:384], cb_sb[:],
                                dwt6[:, it, jh * 384:(jh + 1) * 384],
                                start=True, stop=True)
                        # PSUM->SBUF convert split over both engines:
                        # it 0-2 on DVE with +64*W1 fused, it 3-5 plain
                        # on ACT (W1 restored per-sample in phase 4).
                        dst = mxiall[:, icl, it, :].rearrange(
                            "p (a b) -> p a b", a=2, b=384)
                        if it < 3:
                            w1s = w1i_sb[:, icl, it, :].rearrange(
                                "p (a b) -> p a b", a=2, b=384)
                            nc.vector.tensor_tensor(
                                dst[:], pm2[:, :, 0:384], w1s[:], op=ADD)
                        else:
                            nc.scalar.mul(dst[:], pm2[:, :, 0:384], 1.0)
                    # de-interleave this icl for all samples (overlaps mixing)
                    for b in range(BC):
                        eng = nc.sync if b < 4 else nc.gpsimd
                        eng.dma_start(
                            mxcball[icl * P:(icl + 1) * P, b, :, :],
                            mxiall[b * P:(b + 1) * P, icl, :, :])

            # dw2 loads: issued after the phase-3 de-interleaves on the
            # sync ring, so their transfers overlap phase 4 and land
            # just in time for phase 5.
            dw2_tiles = []
            for t in range(T):
                t_ = dwp.tile([128, KT, D], FP8, tag="dw")
                nc.sync.dma_start(t_[:], dw2[t].rearrange("k p e -> p k e"))
                dw2_tiles.append(t_)

            # ---------- phase 4: final per-sample pass (fp8 normal) ----------
            with tc.tile_pool(name="psF", bufs=4, space="PSUM") as psF:
                for b in range(BC):
                    # restore +64*W1 on the plain-copied it 3..5 slices
                    nc.vector.tensor_tensor(
                        mxcball[:, b, 3:6, :], mxcball[:, b, 3:6, :],
                        w1_sb[:, 3:6, :], op=ADD)
                    ro4 = scr.tile([128, KT, NPAT], BF16, tag="ro4")
                    for jt in range(KT):
                        pf = psF.tile([128, NPAT], F32, tag="f")
                        for it in range(KT):
                            nc.tensor.matmul(
                                pf[:],
                                mxcball[:, b, it, jt * 128:(jt + 1) * 128],
                                xt_sb[:, it, b * NPAT:(b + 1) * NPAT],
                                start=(it == 0), stop=(it == KT - 1))
                        nc.scalar.activation(
                            ro4[:, jt, :], pf[:], RELU,
                            bias=nb1t[:, jt, b:b + 1], scale=1.0 / SC)
                    nc.vector.tensor_reduce(
                        pooln[:, :, b], ro4[:], axis=AXX, op=ADD)

            # ---------- phase 5: layer 2 ----------
            pooln_bf = sm.tile([128, KT, BC], BF16, tag="poolnbf")
            nc.scalar.mul(pooln_bf[:], pooln[:], 1.0 / NPAT)

            # U[(t,kt)][j_local, b] = 64 c[b,t] * pooled[b, .] (fp8, padded)
            uall = sm.tile([128, T, KT, 2 * BC], FP8, tag="uall")
            nc.gpsimd.memset(uall[:], 0.0)
            for t in range(T):
                for kt in range(KT):
                    nc.gpsimd.tensor_tensor(
                        uall[:, t, kt, 0:BC],
                        pooln_bf[:, kt, :],
                        cbc[:, t, :], op=MULT)

            out_sb = sm.tile([BC, D], F32, tag="out")
            tdelt = sm.tile([BC, D], F32, tag="tdelt")
            with tc.tile_pool(name="psV", bufs=4, space="PSUM") as psV:
                po0 = psV.tile([8, 384], F32, tag="v")
                po1 = psV.tile([8, 384], F32, tag="v")
                po = [po0, po1]
                pd0 = psV.tile([16, 384], F32, tag="v8")
                pd1 = psV.tile([16, 384], F32, tag="v8")
                pd = [pd0, pd1]
                # delta chain: DoubleRow over (t, kt-pairs)
                for t in range(T):
                    dwt2 = dw2_tiles[t]
                    for g in range(3):
                        for eh in range(2):
                            nc.tensor.matmul(
                                pd[eh][:],
                                uall[:, t, 2 * g:2 * g + 2, :],
                                dwt2[:, 2 * g:2 * g + 2,
                                     eh * 384:(eh + 1) * 384],
                                start=(t == 0 and g == 0),
                                stop=(t == T - 1 and g == 2),
                                perf_mode=DR)
                # main chain: pooled @ W2 + coefs @ db2 (bf16)
                for eh in range(2):
                    for kt in range(KT):
                        nc.tensor.matmul(
                            po[eh][:], pooln_bf[:, kt, :],
                            w2_sb[:, kt, eh * 384:(eh + 1) * 384],
                            start=(kt == 0), stop=False)
                    nc.tensor.matmul(po[eh][:], coefsT_bf[:],
                                     db2_sb[:, eh * 384:(eh + 1) * 384],
                                     start=False, stop=True)
                    nc.scalar.mul(
                        tdelt[:, eh * 384:(eh + 1) * 384],
                        pd[eh][0:BC, :], 1.0 / (SC * SC))
                    nc.vector.tensor_tensor(
                        out_sb[:, eh * 384:(eh + 1) * 384], po[eh][:],
                        b2r_sb[:, eh * 384:(eh + 1) * 384],
                        op=ADD)
                    nc.vector.tensor_tensor(
                        out_sb[:, eh * 384:(eh + 1) * 384],
                        out_sb[:, eh * 384:(eh + 1) * 384],
                        tdelt[:, eh * 384:(eh + 1) * 384],
                        op=ADD)
                nc.sync.dma_start(out[:], out_sb[:])

    _split_multi_waits(nc)
    return nc


def prep_inputs(x, W1, b1, W2, b2, dW1, db1, dW2, db2, mw1, mb1, mw2, mb2):
    """Host-side layout prep. Returns per-core in_maps."""
    bf = ml_dtypes.bfloat16
    f8 = ml_dtypes.float8_e4m3
    x = np.asarray(x); W1 = np.asarray(W1); W2 = np.asarray(W2)
    b1 = np.asarray(b1); b2 = np.asarray(b2)
    dW1 = np.asarray(dW1); dW2 = np.asarray(dW2)
    db1 = np.asarray(db1); db2 = np.asarray(db2)
    mw1 = np.asarray(mw1); mb1 = np.asarray(mb1)
    mw2 = np.asarray(mw2); mb2 = np.asarray(mb2)

    # patches^T: [B, D, NPAT]
    pt = x.reshape(B, 3, 14, P, 14, P).transpose(0, 1, 3, 5, 2, 4)
    pt = np.ascontiguousarray(pt).reshape(B, D, NPAT)

    # shared (replicated) tensors
    w1_c = np.ascontiguousarray(
        (W1 * SC).reshape(KT, 128, D).transpose(1, 0, 2)).astype(f8)
    # w1i[(b,s), icl, it, j] = 64*W1[(it*8+icl)*16+s, j]  (b-independent)
    w1r = (W1 * SC).reshape(KT, 8, P, D)       # [it, icl, s, j]
    w1i_c = np.ascontiguousarray(np.broadcast_to(
        w1r.transpose(2, 1, 0, 3)[None], (8, P, 8, KT, D)
    ).reshape(128, 8, KT, D)).astype(f8)
    w2_c = np.ascontiguousarray(
        W2.reshape(KT, 128, D).transpose(1, 0, 2)).astype(bf)
    # dw1[icl*6+it, (t,s16), j] = 8*dW1[t, (it*8+icl)*16+s, j]
    d = (dW1 * SCC).reshape(T, KT, 8, P, D)    # [t, it, icl, s, j]
    dw1_c = np.ascontiguousarray(
        d.transpose(2, 1, 0, 3, 4).reshape(8 * KT, 128, D)).astype(f8)
    dw2_c = np.ascontiguousarray(
        (dW2 * SC).reshape(T, KT, 128, D)).astype(f8)
    db1_c = db1.astype(bf)
    db2_c = db2.astype(bf)
    b1t_c = np.ascontiguousarray(b1.reshape(KT, 128).T).astype(np.float32)
    b2t_c = np.ascontiguousarray(b2.reshape(KT, 128).T).astype(np.float32)
    b2r_c = np.tile(b2.astype(np.float32), (BC, 1))
    mw1_c = np.ascontiguousarray(
        mw1.reshape(KT, 128, HM).transpose(1, 0, 2)).astype(bf)
    mb1t_c = np.zeros((128, 2), np.float32)
    mb1t_c[:, 0] = mb1[:128]
    mb1t_c[:64, 1] = mb1[128:]
    mw2_c = np.zeros((128, 2, T), np.float32)
    mw2_c[:, 0, :] = mw2[:128]
    mw2_c[:64, 1, :] = mw2[128:]
    mw2_c = mw2_c.astype(bf)
    mb2t_c = mb2.reshape(T, 1).astype(np.float32)
    iexp_c = np.repeat(np.eye(T, dtype=np.float32), P, axis=1)
    mask16_c = (SCC * np.tile(np.eye(P, dtype=np.float32), (8, 1))).astype(bf)
    selb_c = np.ascontiguousarray(np.broadcast_to(
        (SC * np.eye(T, dtype=np.float32))[:, :, None], (T, T, 128))).astype(bf)

    shared = dict(
        w1=w1_c, w1i=w1i_c, w2=w2_c, dw1=dw1_c, dw2=dw2_c,
        db1=db1_c, db2=db2_c,
        b1t=b1t_c, b2t=b2t_c, b2r=b2r_c, mw1=mw1_c, mb1t=mb1t_c,
        mw2=mw2_c, mb2t=mb2t_c,
        iexp=iexp_c, mask16=mask16_c, selb=selb_c,
    )

    in_maps = []
    for c in range(NCORES):
        ptc = pt[c * BC:(c + 1) * BC]                  # [BC, D, NPAT]
        # xt[p, kt, (b,n)] = ptc[b, kt*128+p, n]
        xt_c = np.ascontiguousarray(
            ptc.reshape(BC, KT, 128, NPAT).transpose(2, 1, 0, 3)
        ).reshape(128, KT, NB).astype(f8)
        m = dict(shared)
        m["xt"] = xt_c
        in_maps.append(m)
    return in_maps


_NC_CACHE = {}


def kernel(**inputs) -> np.ndarray:
    _apply_tile_patch()
    if "nc" not in _NC_CACHE:
        _NC_CACHE["nc"] = build_kernel()
    nc = _NC_CACHE["nc"]
    in_maps = prep_inputs(**inputs)
    res = run_bass_kernel_spmd(nc, in_maps, core_ids=list(range(NCORES)))
    return np.concatenate([r["out"] for r in res.results], axis=0)


# revision 38
# speedup vs baseline: 1.1628x; 1.1628x over previous
"""MetaNetImageEncoder Trainium2 kernel (fp8, v4).

Data-parallel over batch: 8 samples per NeuronCore x 8 cores.

Numerics: x, W1, dW1, dW2, mixed weights nW1 are fp8e4m3. Scales:
  phase 1:  pa = x8 @ (64 W1)                  -> relu(pa/64 + b1)
  phase 3:  pm = (8 c) x (8 dW1) = 64 c dW1;  copy-add w1i(=64 W1)
            -> mxiall = 64 (W1 + M) = 64 nW1   (fp8)
  phase 4:  pf = x8 @ (64 nW1)                 -> relu(pf/64 + nb1)
  phase 5:  po8 = (64 c pooled) @ (64 dW2) = 4096 delta -> * 1/4096
W2, MetaNet, biases, pooled vectors stay bf16/f32.

Engine plan:
  sync    all bulk HBM loads (xt, w1, dw1 x8, w1i, w2, dw2 x8), then
          4 de-interleave DMAs per icl, final store.  No compute.
  scalar  small consts, then pure ACT work (relus) + 2 de-int per icl.
  vector  pool reduces, 3/5 of phase-3 PSUM+w1i add-copies, uall.
  gpsimd  (Pool engine) 2/5 of phase-3 add-copies + 2 de-int per icl.
  tensor  matmuls; phase 1 DoubleRow (FD 392), phase 4 normal fp8
          (FD 196 < 256 so DoubleRow would be LDWEIGHTS-bound),
          phase 5 delta DoubleRow (tiny stationary).
De-interleave DMAs are issued per-icl inside phase 3 so they overlap
the mixing; phase 4 needs no per-sample DMA or W1 add at all.
"""
import numpy as np
import ml_dtypes

import concourse.bass as bass
import concourse.mybir as mybir
import concourse.tile as tile
from concourse.vector_clock import ScopedClock
from concourse.bass_utils import run_bass_kernel_spmd

F32 = mybir.dt.float32
BF16 = mybir.dt.bfloat16
FP8 = mybir.dt.float8e4
DR = mybir.MatmulPerfMode.DoubleRow
RELU = mybir.ActivationFunctionType.Relu
ADD = mybir.AluOpType.add
MULT = mybir.AluOpType.mult
AXX = mybir.AxisListType.X

P = 16
D = 768
T = 8
HM = 192
NPAT = 196          # 14*14 patches
B = 64
NCORES = 8
BC = B // NCORES    # 8 samples per core
NB = BC * NPAT      # 1568
KT = D // 128       # 6 k-tiles
SC = 64.0           # fp8 weight-space scale
SCC = 8.0           # coefficient scale (SCC * (SC/SCC) = SC)

_PATCHED = False
_LDW_OPT = False


def _apply_ldw_opt_patch():
    """walrus is invoked with --enable-ldw-opt=false, which makes every
    matmul pay a serial LDWEIGHTS (~107-213ns).  Rewrite the flag."""
    import concourse.bass_utils as _bu
    if getattr(_bu, "_ldw_patched", False):
        return
    _bu._ldw_patched = True
    _orig = _bu.run_command

    def _patched(cmd, *a, **kw):
        if isinstance(cmd, list):
            cmd = ["--enable-ldw-opt=true" if c == "--enable-ldw-opt=false"
                   else c for c in cmd]
        return _orig(cmd, *a, **kw)

    _bu.run_command = _patched


def _apply_tile_patch():
    """This container's walrus allows only one sem wait per instruction;
    TileContext's exit drain attaches one wait per live semaphore. Split
    them onto standalone single-wait nops."""
    global _PATCHED
    if _PATCHED:
        return
    _PATCHED = True

    def _patched(self, tick_clock, wait_clock):
        carrier = self.nc.sync.nop(nofuse=True, hint="drain_waits")
        wait_clock.add_sem_waits(
            carrier.ins, ScopedClock({None: tick_clock.global_clock})
        )
        si = carrier.ins.sync_info
        waits = list(si.on_wait) if si else []
        if len(waits) > 1:
            carrier.ins.sync_info = mybir.SyncInfo(on_wait=[waits[0]], on_update=[])
            for w in waits[1:]:
                extra = self.nc.sync.nop(nofuse=True, hint="drain_waits")
                extra.ins.sync_info = mybir.SyncInfo(on_wait=[w], on_update=[])
        self.nc.sync.drain()
        self.nc.all_engine_barrier()
        popped = self.nc._tile_sem_poison_stack.pop()
        assert popped is self._sem_poison
        self.nc.clear_and_free_semaphores(list(self.sems.allocated().values()))
        self.nc.all_engine_barrier()

    tile.TileContext._drain_and_barrier = _patched


def _split_multi_waits(nc, max_waits: int = 1):
    """Hoist extra sem waits onto same-engine InstNoOp carriers."""
    for f in nc.m.functions:
        for blk in f.blocks:
            out = []
            for inst in blk.instructions:
                si = inst.sync_info
                if si is not None and len(si.on_wait) > max_waits:
                    waits = list(si.on_wait)
                    for i, w in enumerate(waits[:-max_waits]):
                        out.append(mybir.InstNoOp(
                            name=f"{inst.name}-w{i}",
                            sync_info=mybir.SyncInfo(on_wait=[w], on_update=[]),
                            bass_nofuse=True,
                            engine=inst.engine,
                        ))
                    inst.sync_info = mybir.SyncInfo(
                        on_wait=waits[-max_waits:], on_update=list(si.on_update)
                    )
                out.append(inst)
            blk.instructions = out


def build_kernel():
    nc = bass.Bass(target_bir_lowering=False, trn_type="TRN2")

    din = {}
    def inp(name, shape, dt):
        din[name] = nc.dram_tensor(name, shape, dt, kind="ExternalInput")
        return din[name]

    xt = inp("xt", (128, KT, NB), FP8)           # patches^T  [i_local, kt, (b,n)]
    w1 = inp("w1", (128, KT, D), FP8)            # 64*W1 [i_local, kt, j]
    w1i = inp("w1i", (128, 8, KT, D), FP8)       # 64*W1 [(b,s), icl, it, j]
    w2 = inp("w2", (128, KT, D), BF16)           # W2 [j_local, kt, e]
    dw1 = inp("dw1", (48, 128, D), FP8)          # 8*dW1 [icl*6+it, (t,s16), j]
    dw2 = inp("dw2", (T, KT, 128, D), FP8)       # 64*dW2 [t, kt, j_local, e]
    db1 = inp("db1", (T, D), BF16)
    db2 = inp("db2", (T, D), BF16)
    b1t = inp("b1t", (128, KT), F32)             # b1 [j_local, jt]
    b2t = inp("b2t", (128, KT), F32)             # b2 [e_local, et]
    b2r = inp("b2r", (BC, D), F32)               # b2 replicated over samples
    mw1 = inp("mw1", (128, KT, HM), BF16)
    mb1t = inp("mb1t", (128, 2), F32)
    mw2 = inp("mw2", (128, 2, T), BF16)          # [h_local, g, t], g=1 padded
    mb2t = inp("mb2t", (T, 1), F32)
    iexp = inp("iexp", (T, 128), F32)            # repeat(eye(8),16,axis=1)
    mask16 = inp("mask16", (128, P), BF16)       # 8 * (p%16==s')
    selb = inp("selb", (T, T, 128), BF16)        # 64 * (t'==t)
    maskw = inp("maskw", (128, 128), FP8)        # 0.125 * (p%16==q%16)

    out = nc.dram_tensor("out", (BC, D), F32, kind="ExternalOutput")

    with tile.TileContext(nc) as tc:
        with (
            tc.tile_pool(name="big", bufs=1) as big,
            tc.tile_pool(name="sm", bufs=1) as sm,
            tc.tile_pool(name="dwp", bufs=10) as dwp,
            tc.tile_pool(name="scr", bufs=2) as scr,
        ):
            # ---------- bulk loads: sync ring only (FIFO orders HBM) -----
            # w1 first, then xt in k-chunks: phase 1 starts on chunk 0.
            w1_sb = big.tile([128, KT, D], FP8, tag="w1")
            nc.sync.dma_start(w1_sb[:], w1[:])
            xt_sb = big.tile([128, KT, NB], FP8, tag="xt")
            for g in range(3):
                nc.sync.dma_start(xt_sb[:, 2 * g:2 * g + 2, :],
                                  xt[:, 2 * g:2 * g + 2, :])
            # w2 right after (phase 2 gate), then dw1, then w1i per-icl
            w2_sb = big.tile([128, KT, D], BF16, tag="w2")
            nc.sync.dma_start(w2_sb[:], w2[:])
            dw1_tiles = []
            for icl in range(8):
                t_ = dwp.tile([128, KT, D], FP8, tag="dw")
                nc.sync.dma_start(
                    t_[:],
                    dw1[icl * KT:(icl + 1) * KT].rearrange("k p j -> p k j"))
                dw1_tiles.append(t_)
            w1i_sb = big.tile([128, 8, KT, D], FP8, tag="w1i")
            for icl in range(8):
                nc.sync.dma_start(w1i_sb[:, icl, :, :], w1i[:, icl, :, :])

            # ---------- small consts: scalar ring ----------
            b1t_sb = sm.tile([128, KT], F32, tag="b1t")
            nc.scalar.dma_start(b1t_sb[:], b1t[:])
            mb1t_sb = sm.tile([128, 2], F32, tag="mb1t")
            nc.scalar.dma_start(mb1t_sb[:], mb1t[:])
            mw1_sb = sm.tile([128, KT, HM], BF16, tag="mw1")
            nc.scalar.dma_start(mw1_sb[:], mw1[:])
            mw2_sb = sm.tile([128, 2, T], BF16, tag="mw2")
            nc.scalar.dma_start(mw2_sb[:], mw2[:])
            mb2t_sb = sm.tile([T, 1], F32, tag="mb2t")
            nc.scalar.dma_start(mb2t_sb[:], mb2t[:])
            b2t_sb = sm.tile([128, KT], F32, tag="b2t")
            nc.scalar.dma_start(b2t_sb[:], b2t[:])
            b2r_sb = sm.tile([BC, D], F32, tag="b2r")
            nc.scalar.dma_start(b2r_sb[:], b2r[:])
            iexp_sb = sm.tile([T, 128], F32, tag="iexp")
            nc.scalar.dma_start(iexp_sb[:], iexp[:])
            mask16_sb = sm.tile([128, P], BF16, tag="mask16")
            nc.scalar.dma_start(mask16_sb[:], mask16[:])
            selb_sb = sm.tile([T, T, 128], BF16, tag="selb")
            nc.scalar.dma_start(selb_sb[:], selb[:])
            maskw_sb = sm.tile([128, 128], FP8, tag="maskw")
            nc.scalar.dma_start(maskw_sb[:], maskw[:])
            db1_sb = sm.tile([T, D], BF16, tag="db1")
            nc.scalar.dma_start(db1_sb[:], db1[:])
            db2_sb = sm.tile([T, D], BF16, tag="db2")
            nc.scalar.dma_start(db2_sb[:], db2[:])

            poolb = sm.tile([128, KT, BC], F32, tag="poolb")
            pooln = sm.tile([128, KT, BC], F32, tag="pooln")

            # ---------- phase 1: base pass (fp8 DoubleRow) ----------
            # g outer / ch inner reuses each stationary across 4 matmuls
            with tc.tile_pool(name="psA", bufs=8, space="PSUM") as psA:
                for jt in range(KT):
                    pas = []
                    for ch in range(4):
                        pa = psA.tile([128, 2, NPAT], F32, tag="a")
                        pas.append(pa)
                    for g in range(3):
                        for ch in range(4):
                            nc.tensor.matmul(
                                pas[ch][:].rearrange("p b n -> p (b n)"),
                                w1_sb[:, 2 * g:2 * g + 2,
                                      jt * 128:(jt + 1) * 128],
                                xt_sb[:, 2 * g:2 * g + 2,
                                      ch * 392:(ch + 1) * 392],
                                start=(g == 0), stop=(g == 2),
                                perf_mode=DR)
                    ro = scr.tile([128, 2 * 4, NPAT], BF16, tag="ro")
                    for ch in range(4):
                        nc.scalar.activation(
                            ro[:, 2 * ch:2 * ch + 2, :], pas[ch][:], RELU,
                            bias=b1t_sb[:, jt:jt + 1], scale=1.0 / SC)
                    nc.vector.tensor_reduce(
                        poolb[:, jt, :], ro[:], axis=AXX, op=ADD)

            # ---------- phase 2: MetaNet (bf16) ----------
            with (
                tc.tile_pool(name="pst", bufs=4, space="PSUM") as pst,
                tc.tile_pool(name="psJ", bufs=2, space="PSUM") as psJ,
            ):
                def warm(n=2):
                    for _ in range(n):
                        pj = psJ.tile([128, 512], F32, tag="j")
                        nc.tensor.matmul(
                            pj[:], w1_sb[:, 0, 0:128], xt_sb[:, 0, 0:512],
                            start=True, stop=True)

                poolb_bf = sm.tile([128, KT, BC], BF16, tag="poolbbf")
                nc.scalar.mul(poolb_bf[:], poolb[:], 1.0 / NPAT)

                # base2^T[e, b] = W2.T @ pooled + b2  (input to MetaNet)
                base2_bf = sm.tile([128, KT * BC], BF16, tag="base2bf")
                for et in range(KT):
                    p2 = pst.tile([128, T], F32, tag="tiny")
                    for kt in range(KT):
                        nc.tensor.matmul(
                            p2[:], w2_sb[:, kt, et * 128:(et + 1) * 128],
                            poolb_bf[:, kt, :],
                            start=(kt == 0), stop=(kt == KT - 1))
                    nc.vector.tensor_scalar_add(
                        base2_bf[:, et * BC:(et + 1) * BC], p2[:],
                        b2t_sb[:, et:et + 1])
                warm()

                mh0 = sm.tile([128, T], BF16, tag="mh0")
                mh1 = sm.tile([64, T], BF16, tag="mh1")
                for g, mh_g in ((0, mh0), (1, mh1)):
                    cols = 128 if g == 0 else 64
                    pm = pst.tile([cols, T], F32, tag="tiny")
                    for kt in range(KT):
                        nc.tensor.matmul(
                            pm[:], mw1_sb[:, kt, g * 128:g * 128 + cols],
                            base2_bf[:, kt * BC:(kt + 1) * BC],
                            start=(kt == 0), stop=(kt == KT - 1))
                    nc.scalar.activation(mh_g[:], pm[:], RELU,
                                         bias=mb1t_sb[:cols, g:g + 1])
                warm()

                pc = pst.tile([T, T], F32, tag="tiny")
                nc.tensor.matmul(pc[:], mw2_sb[:, 0, :], mh0[:],
                                 start=True, stop=False)
                nc.tensor.matmul(pc[:], mw2_sb[0:64, 1, :], mh1[:],
                                 start=False, stop=True)
                coefsT = sm.tile([T, T], F32, tag="coefsT")
                nc.vector.tensor_scalar_add(coefsT[:], pc[:], mb2t_sb[:])
                coefsT_bf = sm.tile([T, T], BF16, tag="coefsTbf")
                nc.vector.tensor_copy(coefsT_bf[:], coefsT[:])
                warm()

                # coefficient replication [128, 8]: cRep[(t,s), b] = c[t, b]
                pr = pst.tile([128, T], F32, tag="tiny")
                nc.tensor.matmul(pr[:], iexp_sb[:], coefsT[:],
                                 start=True, stop=True)
                crep = sm.tile([128, T], F32, tag="crep")
                nc.vector.tensor_copy(crep[:], pr[:])

                # block-diag stationary Cb[(t,s), (b,s')] = 8 c (fp8)
                cb_sb = sm.tile([128, 128], FP8, tag="cb")
                for b in range(BC):
                    nc.vector.tensor_scalar_mul(
                        cb_sb[:, b * P:(b + 1) * P], mask16_sb[:],
                        crep[:, b:b + 1])

                # cbc[p, t, b] = 64 c[b, t] for all partitions (phase 5)
                cbc = sm.tile([128, T, BC], BF16, tag="cbc")
                for t in range(T):
                    pu = pst.tile([128, T], F32, tag="tiny")
                    nc.tensor.matmul(pu[:], selb_sb[:, t, :], coefsT_bf[:],
                                     start=True, stop=True)
                    nc.vector.tensor_copy(cbc[:, t, :], pu[:])
                warm()

                # nb1t[j_local, jt, b] = b1 + coefs @ db1
                nb1t = sm.tile([128, KT, BC], F32, tag="nb1t")
                for jt in range(KT):
                    pb = pst.tile([128, T], F32, tag="tiny")
                    nc.tensor.matmul(pb[:], db1_sb[:, jt * 128:(jt + 1) * 128],
                                     coefsT_bf[:], start=True, stop=True)
                    nc.vector.tensor_scalar_add(
                        nb1t[:, jt, :], pb[:], b1t_sb[:, jt:jt + 1])

            # ---------- phase 3: mixing + W1 fold + de-interleave ----------
            mxiall = big.tile([128, 8, KT, D], FP8, tag="mxiall")
            mxcball = big.tile([128, BC, KT, D], FP8, tag="mxcball")
            with tc.tile_pool(name="psM", bufs=4, space="PSUM") as psM:
                for icl in range(8):
                    dwt6 = dw1_tiles[icl]
                    for it in range(KT):
                        pm2 = psM.tile([128, 2, 512], F32, tag="m")  # 2 banks
                        # it 0-3: +64*W1 fused into the DVE copy.
                        # it 4-5: +64*W1 via two extra PE matmuls (maskw
                        # block-diag sums 8 replicas of 8*W1), plain ACT
                        # copy.  Balances PSUM reads across DVE and ACT
                        # while keeping the PE stream dense.
                        fold_pe = it >= 4
                        for jh in range(2):
                            nc.tensor.matmul(
                                pm2[:, jh, 0:384], cb_sb[:],
                                dwt6[:, it, jh * 384:(jh + 1) * 384],
                                start=True, stop=not fold_pe)
                        dst = mxiall[:, icl, it, :].rearrange(
                            "p (a b) -> p a b", a=2, b=384)
                        if fold_pe:
                            for jh in range(2):
                                nc.tensor.matmul(
                                    pm2[:, jh, 0:384], maskw_sb[:],
                                    w1i_sb[:, icl, it,
                                           jh * 384:(jh + 1) * 384],
                                    start=False, stop=True)
                            nc.scalar.mul(dst[:], pm2[:, :, 0:384], 1.0)
                        else:
                            w1s = w1i_sb[:, icl, it, :].rearrange(
                                "p (a b) -> p a b", a=2, b=384)
                            nc.vector.tensor_tensor(
                                dst[:], pm2[:, :, 0:384], w1s[:], op=ADD)
                    # de-interleave this icl for all samples (overlaps mixing)
                    for b in range(BC):
                        eng = (nc.sync if b < 4
                               else nc.scalar if b < 6 else nc.gpsimd)
                        eng.dma_start(
                            mxcball[icl * P:(icl + 1) * P, b, :, :],
                            mxiall[b * P:(b + 1) * P, icl, :, :])

            # dw2 loads: issued after the phase-3 de-interleaves on the
            # sync ring, so their transfers overlap phase 4 and land
            # just in time for phase 5.
            dw2_tiles = []
            for t in range(T):
                t_ = dwp.tile([128, KT, D], FP8, tag="dw")
                nc.sync.dma_start(t_[:], dw2[t].rearrange("k p e -> p k e"))
                dw2_tiles.append(t_)

            # ---------- phase 4: final per-sample pass (fp8 normal) ----------
            with tc.tile_pool(name="psF", bufs=4, space="PSUM") as psF:
                for b in range(BC):
                    ro4 = scr.tile([128, KT, NPAT], BF16, tag="ro4")
                    for jt in range(KT):
                        pf = psF.tile([128, NPAT], F32, tag="f")
                        for it in range(KT):
                            nc.tensor.matmul(
                                pf[:],
                                mxcball[:, b, it, jt * 128:(jt + 1) * 128],
                                xt_sb[:, it, b * NPAT:(b + 1) * NPAT],
                                start=(it == 0), stop=(it == KT - 1))
                        nc.scalar.activation(
                            ro4[:, jt, :], pf[:], RELU,
                            bias=nb1t[:, jt, b:b + 1], scale=1.0 / SC)
                    nc.vector.tensor_reduce(
                        pooln[:, :, b], ro4[:], axis=AXX, op=ADD)

            # ---------- phase 5: layer 2 ----------
            pooln_bf = sm.tile([128, KT, BC], BF16, tag="poolnbf")
            nc.scalar.mul(pooln_bf[:], pooln[:], 1.0 / NPAT)

            # U[(t,kt)][j_local, b] = 64 c[b,t] * pooled[b, .] (fp8, padded)
            uall = sm.tile([128, T, KT, 2 * BC], FP8, tag="uall")
            nc.gpsimd.memset(uall[:], 0.0)
            for t in range(T):
                for kt in range(KT):
                    nc.gpsimd.tensor_tensor(
                        uall[:, t, kt, 0:BC],
                        pooln_bf[:, kt, :],
                        cbc[:, t, :], op=MULT)

            out_sb = sm.tile([BC, D], F32, tag="out")
            tdelt = sm.tile([BC, D], F32, tag="tdelt")
            with tc.tile_pool(name="psV", bufs=4, space="PSUM") as psV:
                po0 = psV.tile([8, 384], F32, tag="v")
                po1 = psV.tile([8, 384], F32, tag="v")
                po = [po0, po1]
                pd0 = psV.tile([16, 384], F32, tag="v8")
                pd1 = psV.tile([16, 384], F32, tag="v8")
                pd = [pd0, pd1]
                # delta chain: DoubleRow over (t, kt-pairs)
                for t in range(T):
                    dwt2 = dw2_tiles[t]
                    for g in range(3):
                        for eh in range(2):
                            nc.tensor.matmul(
                                pd[eh][:],
                                uall[:, t, 2 * g:2 * g + 2, :],
                                dwt2[:, 2 * g:2 * g + 2,
                                     eh * 384:(eh + 1) * 384],
                                start=(t == 0 and g == 0),
                                stop=(t == T - 1 and g == 2),
                                perf_mode=DR)
                # main chain: pooled @ W2 + coefs @ db2 (bf16)
                for eh in range(2):
                    for kt in range(KT):
                        nc.tensor.matmul(
                            po[eh][:], pooln_bf[:, kt, :],
                            w2_sb[:, kt, eh * 384:(eh + 1) * 384],
                            start=(kt == 0), stop=False)
                    nc.tensor.matmul(po[eh][:], coefsT_bf[:],
                                     db2_sb[:, eh * 384:(eh + 1) * 384],
                                     start=False, stop=True)
                    nc.scalar.mul(
                        tdelt[:, eh * 384:(eh + 1) * 384],
                        pd[eh][0:BC, :], 1.0 / (SC * SC))
                    nc.vector.tensor_tensor(
                        out_sb[:, eh * 384:(eh + 1) * 384], po[eh][:],
                        b2r_sb[:, eh * 384:(eh + 1) * 384],
                        op=ADD)
                    nc.vector.tensor_tensor(
                        out_sb[:, eh * 384:(eh + 1) * 384],
                        out_sb[:, eh * 384:(eh + 1) * 384],
                        tdelt[:, eh * 384:(eh + 1) * 384],
                        op=ADD)
                nc.sync.dma_start(out[:], out_sb[:])

    _split_multi_waits(nc)
    return nc


def prep_inputs(x, W1, b1, W2, b2, dW1, db1, dW2, db2, mw1, mb1, mw2, mb2):
    """Host-side layout prep. Returns per-core in_maps."""
    bf = ml_dtypes.bfloat16
    f8 = ml_dtypes.float8_e4m3
    x = np.asarray(x); W1 = np.asarray(W1); W2 = np.asarray(W2)
    b1 = np.asarray(b1); b2 = np.asarray(b2)
    dW1 = np.asarray(dW1); dW2 = np.asarray(dW2)
    db1 = np.asarray(db1); db2 = np.asarray(db2)
    mw1 = np.asarray(mw1); mb1 = np.asarray(mb1)
    mw2 = np.asarray(mw2); mb2 = np.asarray(mb2)

    # patches^T: [B, D, NPAT]
    pt = x.reshape(B, 3, 14, P, 14, P).transpose(0, 1, 3, 5, 2, 4)
    pt = np.ascontiguousarray(pt).reshape(B, D, NPAT)

    # shared (replicated) tensors
    w1_c = np.ascontiguousarray(
        (W1 * SC).reshape(KT, 128, D).transpose(1, 0, 2)).astype(f8)
    # w1i[(b,s), icl, it, j] = 64*W1[(it*8+icl)*16+s, j]  (b-independent)
    w1r = (W1 * SC).reshape(KT, 8, P, D)       # [it, icl, s, j]
    w1i_c = np.ascontiguousarray(np.broadcast_to(
        w1r.transpose(2, 1, 0, 3)[None], (8, P, 8, KT, D)
    ).reshape(128, 8, KT, D)).astype(f8)
    w2_c = np.ascontiguousarray(
        W2.reshape(KT, 128, D).transpose(1, 0, 2)).astype(bf)
    # dw1[icl*6+it, (t,s16), j] = 8*dW1[t, (it*8+icl)*16+s, j]
    d = (dW1 * SCC).reshape(T, KT, 8, P, D)    # [t, it, icl, s, j]
    dw1_c = np.ascontiguousarray(
        d.transpose(2, 1, 0, 3, 4).reshape(8 * KT, 128, D)).astype(f8)
    dw2_c = np.ascontiguousarray(
        (dW2 * SC).reshape(T, KT, 128, D)).astype(f8)
    db1_c = db1.astype(bf)
    db2_c = db2.astype(bf)
    b1t_c = np.ascontiguousarray(b1.reshape(KT, 128).T).astype(np.float32)
    b2t_c = np.ascontiguousarray(b2.reshape(KT, 128).T).astype(np.float32)
    b2r_c = np.tile(b2.astype(np.float32), (BC, 1))
    mw1_c = np.ascontiguousarray(
        mw1.reshape(KT, 128, HM).transpose(1, 0, 2)).astype(bf)
    mb1t_c = np.zeros((128, 2), np.float32)
    mb1t_c[:, 0] = mb1[:128]
    mb1t_c[:64, 1] = mb1[128:]
    mw2_c = np.zeros((128, 2, T), np.float32)
    mw2_c[:, 0, :] = mw2[:128]
    mw2_c[:64, 1, :] = mw2[128:]
    mw2_c = mw2_c.astype(bf)
    mb2t_c = mb2.reshape(T, 1).astype(np.float32)
    iexp_c = np.repeat(np.eye(T, dtype=np.float32), P, axis=1)
    mask16_c = (SCC * np.tile(np.eye(P, dtype=np.float32), (8, 1))).astype(bf)
    maskw_c = (0.125 * np.tile(np.eye(P, dtype=np.float32), (8, 8))).astype(f8)
    selb_c = np.ascontiguousarray(np.broadcast_to(
        (SC * np.eye(T, dtype=np.float32))[:, :, None], (T, T, 128))).astype(bf)

    shared = dict(
        w1=w1_c, w1i=w1i_c, w2=w2_c, dw1=dw1_c, dw2=dw2_c,
        db1=db1_c, db2=db2_c,
        b1t=b1t_c, b2t=b2t_c, b2r=b2r_c, mw1=mw1_c, mb1t=mb1t_c,
        mw2=mw2_c, mb2t=mb2t_c,
        iexp=iexp_c, mask16=mask16_c, selb=selb_c, maskw=maskw_c,
    )

    in_maps = []
    for c in range(NCORES):
        ptc = pt[c * BC:(c + 1) * BC]                  # [BC, D, NPAT]
        # xt[p, kt, (b,n)] = ptc[b, kt*128+p, n]
        xt_c = np.ascontiguousarray(
            ptc.reshape(BC, KT, 128, NPAT).transpose(2, 1, 0, 3)
        ).reshape(128, KT, NB).astype(f8)
        m = dict(shared)
        m["xt"] = xt_c
        in_maps.append(m)
    return in_maps


_NC_CACHE = {}


def kernel(**inputs) -> np.ndarray:
    _apply_tile_patch()
    if _LDW_OPT:
        _apply_ldw_opt_patch()
    if "nc" not in _NC_CACHE:
        _NC_CACHE["nc"] = build_kernel()
    nc = _NC_CACHE["nc"]
    in_maps = prep_inputs(**inputs)
    res = run_bass_kernel_spmd(nc, in_maps, core_ids=list(range(NCORES)))
    return np.concatenate([r["out"] for r in res.results], axis=0)
